# revision 8
# baseline (speedup 1.0000x reference)
"""ALIF neuron rollout (T=100, B=64, J=512, K=1024) on 8 TRN2 NeuronCores.

Strategy (per core, data-parallel over batch, 8 batches/core):
  1. The synaptic-current recurrence is LINEAR, so it folds into TensorE as
     a [T,T] lower-triangular Toeplitz filter applied to the INPUTS:
       xf = L1s-filter(x);  S[t,k] = xf^T @ (w_eff*j_eff) + L2s @ teacher
     All matmuls run in plain fp32 (4-pass): slower per-op than f32r but
     removes the hi/lo split's Vector/Scalar elementwise cost entirely, and
     the kernel stays DMA-bound.  The weight product w_eff*j_eff runs on
     VectorE over streamed per-batch chunks.
  2. k-mapping k = kc*128 + p: the drive transpose slices hsb contiguously
     ([t, 128k] -> [k, t] per kc chunk), and the spike transpose maps back
     to 512B-contiguous DRAM runs for the output DMA.
  3. Drive is stored t-major [128, T, 64slots] so the scan's per-step ADD
     reads a contiguous [128, 32] slice.
  4. The nonlinear threshold scan runs in the u_t = ab^-t rescaled domain
     (adaptation becomes a pure accumulator Q; membrane carry scaled by
     kappa).  Three VectorE ops per step per batch-group (2 custom DVE +
     1 add).  Q history lands in a linear buffer qh[128, T+1, 32] so spike
     extraction batches 4 steps per op: dq4 = qh[t+1:t+5]-qh[t:t+4] on
     GpSimd, PE-transpose [128,128] -> PSUM, ScalarE Sign(PSUM)->stage.
  5. Batches split in 2 groups of 4: G0's scan is emission-interleaved with
     G1's weight streaming so only G1's scan is an exposed tail.  Spikes
     DMA out from stage in 2 chunks per group.
"""
import numpy as np

import concourse.bass as bass
import concourse.tile as tile
from concourse.bass import _add_dep_helper
from concourse import bacc, mybir
from concourse.bass_utils import run_bass_kernel_spmd

T, B, J, K = 100, 64, 512, 1024
DT = 1.0
NCORES = 8
BLOC = B // NCORES           # 8 batches per core
NKC = 8                      # k chunk count: k = kc*128 + p
NSLOT = BLOC * NKC           # 64 scan slots (b*8 + kc)
NG = 2                       # batch groups
GB = BLOC // NG              # 4 batches per group
GSLOT = NSLOT // NG          # 32 slots per group
F32 = mybir.dt.float32


# ---------------------------------------------------------------------------
# Custom DVE ops (registered into concourse.dve_ops at import time).
def _dve_relu_np(x):
    return np.maximum(np.nan_to_num(x, nan=0.0, posinf=np.inf,
                                    neginf=-np.inf), 0)


def _register_dve(name, spec):
    import concourse.dve_ops as dops
    from concourse.dve_spec import lower, _has_src1
    from concourse.dve_uop import DveOpSpec
    if name in dops._SUB_OPCODE_FOR_NAME:
        return next(op for op in dops.OPS if op.name == name)
    row = dops._CUSTOM_DVE_ROW_BASE + len(dops.OPS)
    assert row < 0x20, "custom DVE row budget exhausted"
    shas = {}
    for ver in ("v3", "v4"):
        s = DveOpSpec(name=name, opcode=row, uops=lower(spec, ver=ver),
                      rd1_en=_has_src1(spec))
        shas[ver] = s.sha(ver)
    op = dops.DveOp(name, spec, subdim=False, uops_sha=shas)
    dops.OPS.append(op)
    dops.CUSTOM_DVE_SPECS[name] = spec
    dops._SUB_OPCODE_FOR_NAME[name] = row
    return op


def _alif_ops():
    from concourse.dve_spec import Spec, Src0, Src1, C0, C1, Zero, relu, select
    carry = _register_dve(
        "ALIF_CARRY",
        Spec(
            body=select(Src0 > Src1, Zero, relu(Src0 + C1) * C0),
            reference=lambda in0, in1, s0, s1, imm2: np.where(
                in0 > in1, np.float32(0.0),
                _dve_relu_np(in0 + s1) * s0).astype(np.float32),
        ),
    )
    qacc = _register_dve(
        "ALIF_QACC",
        Spec(
            body=select(Src0 > Src1, C0, Zero) + Src1,
            reference=lambda in0, in1, s0, s1, imm2: (np.where(
                in0 > in1, np.float32(s0), np.float32(0.0)) + in1
            ).astype(np.float32),
        ),
    )
    return carry, qacc


def _scalar(v, name):
    v = np.asarray(v, np.float64)
    if v.ndim == 0:
        return float(v)
    if np.ptp(v) != 0.0:
        raise NotImplementedError(f"{name} must be uniform for this kernel")
    return float(v.reshape(-1)[0])


def _host_constants(w_teach, tau_mem, tau_adapt, tau_epsp, thr_0, beta_adapt):
    dm = DT / _scalar(tau_mem, "tau_mem")
    dmc = 1.0 - dm
    da = DT / _scalar(tau_adapt, "tau_adapt")
    ab = 1.0 - da
    thr0 = _scalar(thr_0, "thr_0")
    assert thr0 > 0.0, "kernel assumes thr_0 > 0"
    beta = _scalar(beta_adapt, "beta_adapt")
    assert beta > 0.0, "spike extraction via Sign(dq) needs beta_adapt > 0"
    epsp = 1.0 - DT / _scalar(tau_epsp, "tau_epsp")
    wt = _scalar(w_teach, "w_teach")

    u = ab ** (-np.arange(T + 1, dtype=np.float64))      # u_t = ab^-t
    kappa = dmc / ab
    g_bias = thr0 * u[:T]                                # G_t
    c_acc = beta * u[:T]                                 # c_t

    tt_, tau_ = np.meshgrid(np.arange(T), np.arange(T), indexing="ij")
    base = np.where(tau_ <= tt_ - 1,
                    epsp ** np.maximum(tt_ - 1 - tau_, 0), 0.0)
    l1 = (u[:T, None] * dm * base).astype(np.float32)    # [t, tau]
    l2 = (u[:T, None] * dm * wt * base).astype(np.float32)
    l1t = np.ascontiguousarray(l1.T)                     # [tau, t]
    l2t = np.ascontiguousarray(l2.T)

    gcol = (-g_bias).astype(np.float32).reshape(T, 1)    # drive bias: -G_t
    id100 = np.eye(T, dtype=np.float32)
    id128 = np.eye(128, dtype=np.float32)
    return dict(kappa=kappa, g_bias=g_bias, c_acc=c_acc,
                l1t=l1t, l2t=l2t, id100=id100, id128=id128, gcol=gcol)


def build_program(consts):
    """One SPMD program; all 8 cores run it on their own batch shard."""
    kappa = float(consts["kappa"])
    g_bias = consts["g_bias"]
    c_acc = consts["c_acc"]
    CARRY, QACC = _alif_ops()
    nc = bacc.Bacc("TRN2", target_bir_lowering=False, debug=False,
                   num_devices=NCORES)

    x_h = nc.declare_dram_parameter("x", [T, BLOC, J], F32, isOutput=False)
    te_h = nc.declare_dram_parameter("teacher", [T, BLOC, K], F32,
                                     isOutput=False)
    we_h = nc.declare_dram_parameter("w_eff", [BLOC, J, K], F32,
                                     isOutput=False)
    je_h = nc.declare_dram_parameter("j_eff", [BLOC, J, K], F32,
                                     isOutput=False)
    l1_h = nc.declare_dram_parameter("l1t", [T, T], F32, isOutput=False)
    l2_h = nc.declare_dram_parameter("l2t", [T, T], F32, isOutput=False)
    i1_h = nc.declare_dram_parameter("id100", [T, T], F32, isOutput=False)
    i2_h = nc.declare_dram_parameter("id128", [128, 128], F32,
                                     isOutput=False)
    gc_h = nc.declare_dram_parameter("gcol", [T, 1], F32, isOutput=False)
    out_h = nc.declare_dram_parameter("out", [T, BLOC, K], F32, isOutput=True)

    JT = J // 128            # 4 j-tiles

    from contextlib import ExitStack
    with tile.TileContext(nc) as tc, ExitStack() as ctx:
        cpool = ctx.enter_context(tc.tile_pool(name="consts", bufs=1))
        xpool = ctx.enter_context(tc.tile_pool(name="x", bufs=1))
        xfpool = ctx.enter_context(tc.tile_pool(name="xf", bufs=1))
        wpool = ctx.enter_context(tc.tile_pool(name="w", bufs=2))
        jpool = ctx.enter_context(tc.tile_pool(name="j", bufs=2))
        tpool = ctx.enter_context(tc.tile_pool(name="teach", bufs=2))
        hpool = ctx.enter_context(tc.tile_pool(name="h", bufs=2))
        dpool = ctx.enter_context(tc.tile_pool(name="drive", bufs=1))
        qpool = ctx.enter_context(tc.tile_pool(name="qh", bufs=1))
        scpool = ctx.enter_context(tc.tile_pool(name="scan", bufs=2))
        dqpool = ctx.enter_context(tc.tile_pool(name="dq", bufs=2))
        stpool = ctx.enter_context(tc.tile_pool(name="stage", bufs=2))
        ps_h = ctx.enter_context(tc.tile_pool(name="psH", bufs=2,
                                              space="PSUM"))
        ps_t = ctx.enter_context(tc.tile_pool(name="psT", bufs=2,
                                              space="PSUM"))
        ps_s = ctx.enter_context(tc.tile_pool(name="psS", bufs=2,
                                              space="PSUM"))

        # --- prologue DMAs: chained in need-order.  All DMA lanes drain
        # concurrently, so without explicit deps every queued transfer
        # fair-shares bandwidth and COMPLETES late together; chaining batch
        # groups serializes them so completion order == consumption order.
        we_t = [None] * BLOC
        je_t = [None] * BLOC
        te_t = [None] * BLOC
        dma_group = [[], []]      # [current handles, prev group handles]

        def chained_dma(out, in_):
            h = nc.sync.dma_start(out, in_)
            for p in dma_group[1]:
                _add_dep_helper(h.ins, p.ins, sync=True,
                                reason="dma stream order")
            dma_group[0].append(h)
            return h

        def next_dma_group():
            dma_group[1] = dma_group[0]
            dma_group[0] = []

        def issue_b_dmas(b):
            next_dma_group()
            we_t[b] = wpool.tile([128, JT, K], F32, tag="weff", name="weff")
            je_t[b] = jpool.tile([128, JT, K], F32, tag="jeff", name="jeff")
            te_t[b] = tpool.tile([T, K], F32, tag="teach", name="teach")
            we_src = we_h.ap()[b].rearrange("(jt p) k -> p jt k", p=128)
            je_src = je_h.ap()[b].rearrange("(jt p) k -> p jt k", p=128)
            chained_dma(we_t[b][:, :2], we_src[:, :2])
            chained_dma(we_t[b][:, 2:], we_src[:, 2:])
            chained_dma(je_t[b][:, :2], je_src[:, :2])
            chained_dma(je_t[b][:, 2:], je_src[:, 2:])
            chained_dma(te_t[b][:], te_h.ap()[:, b, :])

        l1t_sb = cpool.tile([T, T], F32, tag="l1")
        l2t_sb = cpool.tile([T, T], F32, tag="l2")
        i1_sb = cpool.tile([T, T], F32, tag="id100")
        i2_sb = cpool.tile([128, 128], F32, tag="id128")
        gc_sb = cpool.tile([T, 1], F32, tag="gc")
        x_sb = xpool.tile([T, BLOC, J], F32, tag="x")
        chained_dma(x_sb[:], x_h.ap()[:])
        chained_dma(l1t_sb[:], l1_h.ap()[:])
        chained_dma(l2t_sb[:], l2_h.ap()[:])
        chained_dma(i1_sb[:], i1_h.ap()[:])
        chained_dma(i2_sb[:], i2_h.ap()[:])
        chained_dma(gc_sb[:], gc_h.ap()[:])

        issue_b_dmas(0)
        issue_b_dmas(1)

        # --- x-filter fold: xf[j, t] = sum_tau x[tau, j] * L1s[t, tau]
        xf_sb = xfpool.tile([128, BLOC * JT, T], F32, tag="xf")
        for b in range(BLOC):
            for jt in range(JT):
                xp = ps_t.tile([128, T], F32, tag="pst")
                nc.tensor.matmul(xp[:],
                                 lhsT=x_sb[:, b, jt * 128:(jt + 1) * 128],
                                 rhs=l1t_sb[:], start=True, stop=True)
                nc.scalar.copy(xf_sb[:, b * JT + jt, :], xp[:])

        # --- drive tiles, t-major: [128 (p), 100 (t), 64 (b*8+kc)]
        drive_sb = dpool.tile([128, T, NSLOT], F32, tag="drive")

        # --- scan state (shared across both groups sequentially)
        qh = qpool.tile([128, T + 1, GSLOT], F32, tag="qh")
        stage = [None, None]
        p_prev = [None]

        def emit_spikes(g, t0, nsteps):
            """Extract spikes for steps [t0, t0+nsteps) from the Q history."""
            dq = dqpool.tile([128, 8, GSLOT], F32, tag="dq8", name="dq8")
            nc.vector.tensor_tensor(
                dq[:, :nsteps], qh[:, t0 + 1:t0 + 1 + nsteps, :],
                qh[:, t0:t0 + nsteps, :], mybir.AluOpType.subtract)
            for h in range(nsteps // 4):
                m = t0 // 4 + h
                sps = ps_s.tile([128, 128], F32, tag="spk")
                nc.tensor.transpose(
                    sps[:], dq[:, h * 4:(h + 1) * 4].rearrange(
                        "p t4 s -> p (t4 s)"), i2_sb[:])
                nc.scalar.activation(stage[g][:, m, :], sps[:],
                                     mybir.ActivationFunctionType.Sign)

        def emit_scan_step(g, t):
            """One u-domain threshold-scan step for group g."""
            gsl = slice(g * GSLOT, (g + 1) * GSLOT)
            p_in = drive_sb[:, 0, gsl] if t == 0 else p_prev[0][:]
            if t < T - 1:
                c_t = scpool.tile([128, GSLOT], F32, tag="C")
                nc.vector._custom_dve(
                    CARRY, out=c_t[:], in0=p_in, in1=qh[:, t, :],
                    s0=kappa, s1=float(g_bias[t]))
            nc.vector._custom_dve(
                QACC, out=qh[:, t + 1, :], in0=p_in, in1=qh[:, t, :],
                s0=float(c_acc[t]))
            if t < T - 1:
                p_new = scpool.tile([128, GSLOT], F32, tag="P")
                nc.vector.tensor_tensor(p_new[:], c_t[:],
                                        drive_sb[:, t + 1, gsl],
                                        mybir.AluOpType.add)
                p_prev[0] = p_new
            if t % 8 == 7:
                emit_spikes(g, t - 7, 8)
            elif t == T - 1:
                emit_spikes(g, t - 3, 4)

        def emit_scan_start(g):
            nc.vector.memset(qh[:, 0, :], 0.0)
            stage[g] = stpool.tile([128, T // 4, 128], F32, tag="stage", name="stage")
            p_prev[0] = None

        # out DMA view: t = tq*4 + t4, b = g*4 + bl, k = kc*128 + p
        out_r = out_h.ap().rearrange(
            "(tq t4) (g bl) (kc p) -> t4 g (bl kc) tq p",
            t4=4, bl=GB, p=128)

        def emit_out_dma(g, m0, m1):
            next_dma_group()
            for t4 in range(4):
                chained_dma(out_r[t4, g, :, m0:m1, :],
                            stage[g][t4 * 32:(t4 + 1) * 32, m0:m1, :])

        def emit_b(b, scan_g=None, scan_iter=None):
            """Stream one batch: weight product + matmuls + drive transpose.
            Optionally interleave scan-step emission for group scan_g."""
            if b + 2 < BLOC:
                issue_b_dmas(b + 2)

            def steps(n):
                if scan_iter is None:
                    return
                for _ in range(n):
                    t = next(scan_iter, None)
                    if t is None:
                        return
                    emit_scan_step(scan_g, t)

            # w = w_eff * j_eff in place, 4 chunks of [128, 1024].
            # GpSimd for all but the last batch keeps VectorE free for the
            # scan; the last batch uses VectorE so its drive (which gates the
            # exposed final scan) is not stuck behind GpSimd's queue.
            eng = nc.vector if b < GB else nc.gpsimd
            for jt in range(JT):
                eng.tensor_tensor(
                    we_t[b][:, jt], we_t[b][:, jt], je_t[b][:, jt],
                    mybir.AluOpType.mult)
                steps(7)

            hps = ps_h.tile([T, K], F32, tag="hps")
            hsb = hpool.tile([T, K], F32, tag="hsb")
            for half in range(2):
                ksl = slice(half * 512, (half + 1) * 512)
                for jt in range(JT):
                    nc.tensor.matmul(
                        hps[:, ksl],
                        lhsT=xf_sb[:, b * JT + jt, :],
                        rhs=we_t[b][:, jt, ksl],
                        start=(jt == 0), stop=False)
                nc.tensor.matmul(
                    hps[:, ksl], lhsT=l2t_sb[:], rhs=te_t[b][:, ksl],
                    start=False, stop=True)
                steps(1)
                # d_hat[t] = d[t] - G_t, PSUM -> SBUF with bias
                nc.scalar.activation(hsb[:, ksl], hps[:, ksl],
                                     mybir.ActivationFunctionType.Identity,
                                     bias=gc_sb[:, 0:1], scale=1.0)
                # transpose drive [t, 128k] -> [k, t] per kc chunk
                for kc in range(half * 4, half * 4 + 4):
                    dps = ps_t.tile([128, T], F32, tag="pst")
                    nc.tensor.transpose(
                        dps[:], hsb[:, kc * 128:(kc + 1) * 128], i1_sb[:])
                    nc.scalar.copy(drive_sb[:, :, b * NKC + kc], dps[:])
                    steps(1)
            we_t[b] = None
            je_t[b] = None
            te_t[b] = None

        # --- phase A: group 0 batches, no scan yet
        for b in range(GB):
            emit_b(b)

        # --- phase B: group 1 batches with group-0 scan interleaved
        emit_scan_start(0)
        it0 = iter(range(T))
        for b in range(GB, BLOC):
            emit_b(b, scan_g=0, scan_iter=it0)
        for t in it0:
            emit_scan_step(0, t)
        emit_out_dma(0, 0, 13)

        # --- phase C: group-1 scan + outputs
        emit_scan_start(1)
        for t in range(T):
            emit_scan_step(1, t)
            if t == 55:
                emit_out_dma(0, 13, T // 4)
                emit_out_dma(1, 0, 13)
        emit_out_dma(1, 13, T // 4)

    nc.compile()
    return nc


def _prepare(inputs):
    x = np.ascontiguousarray(np.asarray(inputs["x"], np.float32))
    teacher = np.ascontiguousarray(np.asarray(inputs["teacher"], np.float32))
    w_eff = np.ascontiguousarray(np.asarray(inputs["w_eff"], np.float32))
    j_eff = np.ascontiguousarray(np.asarray(inputs["j_eff"], np.float32))
    consts = _host_constants(
        inputs["w_teach"], inputs["tau_mem"], inputs["tau_adapt"],
        inputs["tau_epsp"], inputs["thr_0"], inputs["beta_adapt"])
    in_maps = []
    for i in range(NCORES):
        sl = slice(i * BLOC, (i + 1) * BLOC)
        in_maps.append({
            "x": np.ascontiguousarray(x[:, sl]),
            "teacher": np.ascontiguousarray(teacher[:, sl]),
            "w_eff": np.ascontiguousarray(w_eff[sl]),
            "j_eff": np.ascontiguousarray(j_eff[sl]),
            "l1t": consts["l1t"], "l2t": consts["l2t"],
            "id100": consts["id100"], "id128": consts["id128"],
            "gcol": consts["gcol"],
        })
    return consts, in_maps


def run(inputs, trace=False, **kw):
    consts, in_maps = _prepare(inputs)
    nc = build_program(consts)
    res = run_bass_kernel_spmd(nc, in_maps, core_ids=list(range(NCORES)),
                               trace=trace, **kw)
    out = np.concatenate([res.results[i]["out"] for i in range(NCORES)],
                         axis=1)
    return out.astype(np.float32), res


def kernel(**inputs) -> np.ndarray:
    out, _ = run(inputs)
    return out


# revision 10
# speedup vs baseline: 1.0763x; 1.0763x over previous
"""ALIF neuron rollout (T=100, B=64, J=512, K=1024) on 8 TRN2 NeuronCores.

Strategy (per core, data-parallel over batch, 8 batches/core):
  1. The synaptic-current recurrence is LINEAR, so it folds into TensorE as
     a [T,T] lower-triangular Toeplitz filter applied to the INPUTS:
       xf = L1s-filter(x);  S[t,k] = xf^T @ (w_eff*j_eff) + L2s @ teacher
     All matmuls run in plain fp32 (4-pass): slower per-op than f32r but
     removes the hi/lo split's Vector/Scalar elementwise cost entirely, and
     the kernel stays DMA-bound.  The weight product w_eff*j_eff runs on
     VectorE over streamed per-batch chunks.
  2. k-mapping k = kc*128 + p: the drive transpose slices hsb contiguously
     ([t, 128k] -> [k, t] per kc chunk), and the spike transpose maps back
     to 512B-contiguous DRAM runs for the output DMA.
  3. Drive is stored t-major [128, T, 64slots] so the scan's per-step ADD
     reads a contiguous [128, 32] slice.
  4. The nonlinear threshold scan runs in the u_t = ab^-t rescaled domain
     (adaptation becomes a pure accumulator Q; membrane carry scaled by
     kappa).  Three VectorE ops per step per batch-group (2 custom DVE +
     1 add).  Q history lands in a linear buffer qh[128, T+1, 32] so spike
     extraction batches 4 steps per op: dq4 = qh[t+1:t+5]-qh[t:t+4] on
     GpSimd, PE-transpose [128,128] -> PSUM, ScalarE Sign(PSUM)->stage.
  5. Batches split in 2 groups of 4: G0's scan is emission-interleaved with
     G1's weight streaming so only G1's scan is an exposed tail.  Spikes
     DMA out from stage in 2 chunks per group.
"""
import numpy as np

import concourse.bass as bass
import concourse.tile as tile
from concourse.bass import _add_dep_helper
from concourse import bacc, mybir
from concourse.bass_utils import run_bass_kernel_spmd

T, B, J, K = 100, 64, 512, 1024
DT = 1.0
NCORES = 8
BLOC = B // NCORES           # 8 batches per core
NKC = 8                      # k chunk count: k = kc*128 + p
NSLOT = BLOC * NKC           # 64 scan slots (b*8 + kc)
NG = 2                       # batch groups
GB = BLOC // NG              # 4 batches per group
GSLOT = NSLOT // NG          # 32 slots per group
F32 = mybir.dt.float32


# ---------------------------------------------------------------------------
# Custom DVE ops (registered into concourse.dve_ops at import time).
def _dve_relu_np(x):
    return np.maximum(np.nan_to_num(x, nan=0.0, posinf=np.inf,
                                    neginf=-np.inf), 0)


def _register_dve(name, spec):
    import concourse.dve_ops as dops
    from concourse.dve_spec import lower, _has_src1
    from concourse.dve_uop import DveOpSpec
    if name in dops._SUB_OPCODE_FOR_NAME:
        return next(op for op in dops.OPS if op.name == name)
    row = dops._CUSTOM_DVE_ROW_BASE + len(dops.OPS)
    assert row < 0x20, "custom DVE row budget exhausted"
    shas = {}
    for ver in ("v3", "v4"):
        s = DveOpSpec(name=name, opcode=row, uops=lower(spec, ver=ver),
                      rd1_en=_has_src1(spec))
        shas[ver] = s.sha(ver)
    op = dops.DveOp(name, spec, subdim=False, uops_sha=shas)
    dops.OPS.append(op)
    dops.CUSTOM_DVE_SPECS[name] = spec
    dops._SUB_OPCODE_FOR_NAME[name] = row
    return op


def _alif_ops():
    from concourse.dve_spec import Spec, Src0, Src1, C0, C1, Zero, relu, select
    carry = _register_dve(
        "ALIF_CARRY",
        Spec(
            body=select(Src0 > Src1, Zero, relu(Src0 + C1) * C0),
            reference=lambda in0, in1, s0, s1, imm2: np.where(
                in0 > in1, np.float32(0.0),
                _dve_relu_np(in0 + s1) * s0).astype(np.float32),
        ),
    )
    qacc = _register_dve(
        "ALIF_QACC",
        Spec(
            body=select(Src0 > Src1, C0, Zero) + Src1,
            reference=lambda in0, in1, s0, s1, imm2: (np.where(
                in0 > in1, np.float32(s0), np.float32(0.0)) + in1
            ).astype(np.float32),
        ),
    )
    return carry, qacc


def _scalar(v, name):
    v = np.asarray(v, np.float64)
    if v.ndim == 0:
        return float(v)
    if np.ptp(v) != 0.0:
        raise NotImplementedError(f"{name} must be uniform for this kernel")
    return float(v.reshape(-1)[0])


def _host_constants(w_teach, tau_mem, tau_adapt, tau_epsp, thr_0, beta_adapt):
    dm = DT / _scalar(tau_mem, "tau_mem")
    dmc = 1.0 - dm
    da = DT / _scalar(tau_adapt, "tau_adapt")
    ab = 1.0 - da
    thr0 = _scalar(thr_0, "thr_0")
    assert thr0 > 0.0, "kernel assumes thr_0 > 0"
    beta = _scalar(beta_adapt, "beta_adapt")
    assert beta > 0.0, "spike extraction via Sign(dq) needs beta_adapt > 0"
    epsp = 1.0 - DT / _scalar(tau_epsp, "tau_epsp")
    wt = _scalar(w_teach, "w_teach")

    u = ab ** (-np.arange(T + 1, dtype=np.float64))      # u_t = ab^-t
    kappa = dmc / ab
    g_bias = thr0 * u[:T]                                # G_t
    c_acc = beta * u[:T]                                 # c_t

    tt_, tau_ = np.meshgrid(np.arange(T), np.arange(T), indexing="ij")
    base = np.where(tau_ <= tt_ - 1,
                    epsp ** np.maximum(tt_ - 1 - tau_, 0), 0.0)
    l1 = (u[:T, None] * dm * base).astype(np.float32)    # [t, tau]
    l2 = (u[:T, None] * dm * wt * base).astype(np.float32)
    l1t = np.ascontiguousarray(l1.T)                     # [tau, t]
    l2t = np.ascontiguousarray(l2.T)

    gcol = (-g_bias).astype(np.float32).reshape(T, 1)    # drive bias: -G_t
    id100 = np.eye(T, dtype=np.float32)
    id128 = np.eye(128, dtype=np.float32)
    return dict(kappa=kappa, g_bias=g_bias, c_acc=c_acc,
                l1t=l1t, l2t=l2t, id100=id100, id128=id128, gcol=gcol)


def build_program(consts):
    """One SPMD program; all 8 cores run it on their own batch shard."""
    kappa = float(consts["kappa"])
    g_bias = consts["g_bias"]
    c_acc = consts["c_acc"]
    CARRY, QACC = _alif_ops()
    nc = bacc.Bacc("TRN2", target_bir_lowering=False, debug=False,
                   num_devices=NCORES)

    x_h = nc.declare_dram_parameter("x", [T, BLOC, J], F32, isOutput=False)
    te_h = nc.declare_dram_parameter("teacher", [T, BLOC, K], F32,
                                     isOutput=False)
    we_h = nc.declare_dram_parameter("w_eff", [BLOC, J, K], F32,
                                     isOutput=False)
    je_h = nc.declare_dram_parameter("j_eff", [BLOC, J, K], F32,
                                     isOutput=False)
    l1_h = nc.declare_dram_parameter("l1t", [T, T], F32, isOutput=False)
    l2_h = nc.declare_dram_parameter("l2t", [T, T], F32, isOutput=False)
    i1_h = nc.declare_dram_parameter("id100", [T, T], F32, isOutput=False)
    i2_h = nc.declare_dram_parameter("id128", [128, 128], F32,
                                     isOutput=False)
    gc_h = nc.declare_dram_parameter("gcol", [T, 1], F32, isOutput=False)
    out_h = nc.declare_dram_parameter("out", [T, BLOC, K], F32, isOutput=True)

    JT = J // 128            # 4 j-tiles

    from contextlib import ExitStack
    with tile.TileContext(nc) as tc, ExitStack() as ctx:
        cpool = ctx.enter_context(tc.tile_pool(name="consts", bufs=1))
        xpool = ctx.enter_context(tc.tile_pool(name="x", bufs=1))
        xfpool = ctx.enter_context(tc.tile_pool(name="xf", bufs=1))
        wpool = ctx.enter_context(tc.tile_pool(name="w", bufs=3))
        jpool = ctx.enter_context(tc.tile_pool(name="j", bufs=2))
        tpool = ctx.enter_context(tc.tile_pool(name="teach", bufs=2))
        hpool = ctx.enter_context(tc.tile_pool(name="h", bufs=2))
        dpool = ctx.enter_context(tc.tile_pool(name="drive", bufs=1))
        qpool = ctx.enter_context(tc.tile_pool(name="qh", bufs=1))
        scpool = ctx.enter_context(tc.tile_pool(name="scan", bufs=2))
        dqpool = ctx.enter_context(tc.tile_pool(name="dq", bufs=2))
        stpool = ctx.enter_context(tc.tile_pool(name="stage", bufs=1))
        ps_h = ctx.enter_context(tc.tile_pool(name="psH", bufs=2,
                                              space="PSUM"))
        ps_t = ctx.enter_context(tc.tile_pool(name="psT", bufs=2,
                                              space="PSUM"))
        ps_s = ctx.enter_context(tc.tile_pool(name="psS", bufs=2,
                                              space="PSUM"))

        # --- prologue DMAs: chained in need-order.  All DMA lanes drain
        # concurrently, so without explicit deps every queued transfer
        # fair-shares bandwidth and COMPLETES late together; chaining batch
        # groups serializes them so completion order == consumption order.
        we_t = [None] * BLOC
        je_t = [None] * BLOC
        te_t = [None] * BLOC
        dma_chain = []            # sliding-window ordering of stream DMAs
        CHAIN_W = 4               # in-flight transfers (~3-4 MB rolling)

        def chained_dma(out, in_):
            h = nc.sync.dma_start(out, in_)
            if len(dma_chain) >= CHAIN_W:
                _add_dep_helper(h.ins, dma_chain[-CHAIN_W].ins, sync=True,
                                reason="dma stream order")
            dma_chain.append(h)
            return h

        def issue_b_dmas(b):
            we_t[b] = wpool.tile([128, JT, K], F32, tag="weff", name="weff")
            je_t[b] = jpool.tile([128, JT, K], F32, tag="jeff", name="jeff")
            te_t[b] = tpool.tile([T, K], F32, tag="teach", name="teach")
            we_src = we_h.ap()[b].rearrange("(jt p) k -> p jt k", p=128)
            je_src = je_h.ap()[b].rearrange("(jt p) k -> p jt k", p=128)
            chained_dma(we_t[b][:, :2], we_src[:, :2])
            chained_dma(je_t[b][:, :2], je_src[:, :2])
            chained_dma(we_t[b][:, 2:], we_src[:, 2:])
            chained_dma(je_t[b][:, 2:], je_src[:, 2:])
            chained_dma(te_t[b][:], te_h.ap()[:, b, :])

        l1t_sb = cpool.tile([T, T], F32, tag="l1")
        l2t_sb = cpool.tile([T, T], F32, tag="l2")
        i1_sb = cpool.tile([T, T], F32, tag="id100")
        i2_sb = cpool.tile([128, 128], F32, tag="id128")
        gc_sb = cpool.tile([T, 1], F32, tag="gc")
        x_sb = xpool.tile([T, BLOC, J], F32, tag="x")
        chained_dma(x_sb[:], x_h.ap()[:])
        chained_dma(l1t_sb[:], l1_h.ap()[:])

        issue_b_dmas(0)

        chained_dma(l2t_sb[:], l2_h.ap()[:])
        chained_dma(i1_sb[:], i1_h.ap()[:])
        chained_dma(i2_sb[:], i2_h.ap()[:])
        chained_dma(gc_sb[:], gc_h.ap()[:])

        issue_b_dmas(1)

        # --- x-filter fold: xf[j, t] = sum_tau x[tau, j] * L1s[t, tau]
        xf_sb = xfpool.tile([128, BLOC * JT, T], F32, tag="xf")
        for b in range(BLOC):
            for jt in range(JT):
                xp = ps_t.tile([128, T], F32, tag="pst")
                nc.tensor.matmul(xp[:],
                                 lhsT=x_sb[:, b, jt * 128:(jt + 1) * 128],
                                 rhs=l1t_sb[:], start=True, stop=True)
                nc.scalar.copy(xf_sb[:, b * JT + jt, :], xp[:])

        # --- drive tiles, t-major: [128 (p), 100 (t), 64 (b*8+kc)]
        drive_sb = dpool.tile([128, T, NSLOT], F32, tag="drive")

        # --- scan state (shared across both groups sequentially)
        qh = qpool.tile([128, T + 1, GSLOT], F32, tag="qh")
        stage = [None, None]
        p_prev = [None]

        def emit_spikes(g, t0, nsteps):
            """Extract spikes for steps [t0, t0+nsteps) from the Q history."""
            dq = dqpool.tile([128, 8, GSLOT], F32, tag="dq8", name="dq8")
            nc.vector.tensor_tensor(
                dq[:, :nsteps], qh[:, t0 + 1:t0 + 1 + nsteps, :],
                qh[:, t0:t0 + nsteps, :], mybir.AluOpType.subtract)
            for h in range(nsteps // 4):
                m = t0 // 4 + h
                sps = ps_s.tile([128, 128], F32, tag="spk")
                nc.tensor.transpose(
                    sps[:], dq[:, h * 4:(h + 1) * 4].rearrange(
                        "p t4 s -> p (t4 s)"), i2_sb[:])
                nc.scalar.activation(stage[g][:, m, :], sps[:],
                                     mybir.ActivationFunctionType.Sign)

        def emit_scan_step(g, t):
            """One u-domain threshold-scan step for group g."""
            gsl = slice(g * GSLOT, (g + 1) * GSLOT)
            p_in = drive_sb[:, 0, gsl] if t == 0 else p_prev[0][:]
            if t < T - 1:
                c_t = scpool.tile([128, GSLOT], F32, tag="C")
                nc.vector._custom_dve(
                    CARRY, out=c_t[:], in0=p_in, in1=qh[:, t, :],
                    s0=kappa, s1=float(g_bias[t]))
            nc.vector._custom_dve(
                QACC, out=qh[:, t + 1, :], in0=p_in, in1=qh[:, t, :],
                s0=float(c_acc[t]))
            if t < T - 1:
                p_new = scpool.tile([128, GSLOT], F32, tag="P")
                nc.vector.tensor_tensor(p_new[:], c_t[:],
                                        drive_sb[:, t + 1, gsl],
                                        mybir.AluOpType.add)
                p_prev[0] = p_new
            if t % 8 == 7:
                emit_spikes(g, t - 7, 8)
            elif t == T - 1:
                emit_spikes(g, t - 3, 4)

        def emit_scan_start(g):
            nc.vector.memset(qh[:, 0, :], 0.0)
            stage[g] = stpool.tile([128, T // 4, 128], F32, tag="stage", name="stage")
            p_prev[0] = None

        # out DMA view: t = tq*4 + t4, b = g*4 + bl, k = kc*128 + p
        out_r = out_h.ap().rearrange(
            "(tq t4) (g bl) (kc p) -> t4 g (bl kc) tq p",
            t4=4, bl=GB, p=128)

        def emit_out_dma(g, m0, m1):
            for t4 in range(4):
                chained_dma(out_r[t4, g, :, m0:m1, :],
                            stage[g][t4 * 32:(t4 + 1) * 32, m0:m1, :])

        def emit_b(b, scan_g=None, scan_iter=None):
            """Stream one batch: weight product + matmuls + drive transpose.
            Optionally interleave scan-step emission for group scan_g."""
            if b + 2 < BLOC:
                issue_b_dmas(b + 2)

            def steps(n):
                if scan_iter is None:
                    return
                for _ in range(n):
                    t = next(scan_iter, None)
                    if t is None:
                        return
                    emit_scan_step(scan_g, t)

            # w = w_eff * j_eff in place, 4 chunks of [128, 1024].
            # GpSimd for all but the last batch keeps VectorE free for the
            # scan; the last batch uses VectorE so its drive (which gates the
            # exposed final scan) is not stuck behind GpSimd's queue.
            eng = nc.vector if b < GB else nc.gpsimd
            for jt in range(JT):
                eng.tensor_tensor(
                    we_t[b][:, jt], we_t[b][:, jt], je_t[b][:, jt],
                    mybir.AluOpType.mult)
                steps(7)

            hps = ps_h.tile([T, K], F32, tag="hps")
            hsb = hpool.tile([T, K], F32, tag="hsb")
            for half in range(2):
                ksl = slice(half * 512, (half + 1) * 512)
                for jt in range(JT):
                    nc.tensor.matmul(
                        hps[:, ksl],
                        lhsT=xf_sb[:, b * JT + jt, :],
                        rhs=we_t[b][:, jt, ksl],
                        start=(jt == 0), stop=False)
                nc.tensor.matmul(
                    hps[:, ksl], lhsT=l2t_sb[:], rhs=te_t[b][:, ksl],
                    start=False, stop=True)
                steps(1)
                # d_hat[t] = d[t] - G_t, PSUM -> SBUF with bias
                nc.scalar.activation(hsb[:, ksl], hps[:, ksl],
                                     mybir.ActivationFunctionType.Identity,
                                     bias=gc_sb[:, 0:1], scale=1.0)
                # transpose drive [t, 128k] -> [k, t] per kc chunk
                for kc in range(half * 4, half * 4 + 4):
                    dps = ps_t.tile([128, T], F32, tag="pst")
                    nc.tensor.transpose(
                        dps[:], hsb[:, kc * 128:(kc + 1) * 128], i1_sb[:])
                    nc.scalar.copy(drive_sb[:, :, b * NKC + kc], dps[:])
                    steps(1)
            we_t[b] = None
            je_t[b] = None
            te_t[b] = None

        # --- phase A: group 0 batches, no scan yet
        for b in range(GB):
            emit_b(b)

        # --- phase B: group 1 batches with group-0 scan interleaved
        emit_scan_start(0)
        it0 = iter(range(T))
        for b in range(GB, BLOC):
            emit_b(b, scan_g=0, scan_iter=it0)
        for t in it0:
            emit_scan_step(0, t)
        emit_out_dma(0, 0, 13)
        emit_out_dma(0, 13, T // 4)

        # --- phase C: group-1 scan + outputs
        emit_scan_start(1)
        for t in range(T):
            emit_scan_step(1, t)
            if t == 60:
                emit_out_dma(1, 0, 13)
        emit_out_dma(1, 13, T // 4)

    nc.compile()
    return nc


def _prepare(inputs):
    x = np.ascontiguousarray(np.asarray(inputs["x"], np.float32))
    teacher = np.ascontiguousarray(np.asarray(inputs["teacher"], np.float32))
    w_eff = np.ascontiguousarray(np.asarray(inputs["w_eff"], np.float32))
    j_eff = np.ascontiguousarray(np.asarray(inputs["j_eff"], np.float32))
    consts = _host_constants(
        inputs["w_teach"], inputs["tau_mem"], inputs["tau_adapt"],
        inputs["tau_epsp"], inputs["thr_0"], inputs["beta_adapt"])
    in_maps = []
    for i in range(NCORES):
        sl = slice(i * BLOC, (i + 1) * BLOC)
        in_maps.append({
            "x": np.ascontiguousarray(x[:, sl]),
            "teacher": np.ascontiguousarray(teacher[:, sl]),
            "w_eff": np.ascontiguousarray(w_eff[sl]),
            "j_eff": np.ascontiguousarray(j_eff[sl]),
            "l1t": consts["l1t"], "l2t": consts["l2t"],
            "id100": consts["id100"], "id128": consts["id128"],
            "gcol": consts["gcol"],
        })
    return consts, in_maps


def run(inputs, trace=False, **kw):
    consts, in_maps = _prepare(inputs)
    nc = build_program(consts)
    res = run_bass_kernel_spmd(nc, in_maps, core_ids=list(range(NCORES)),
                               trace=trace, **kw)
    out = np.concatenate([res.results[i]["out"] for i in range(NCORES)],
                         axis=1)
    return out.astype(np.float32), res


def kernel(**inputs) -> np.ndarray:
    out, _ = run(inputs)
    return out


# revision 11
# speedup vs baseline: 1.0877x; 1.0106x over previous
"""ALIF neuron rollout (T=100, B=64, J=512, K=1024) on 8 TRN2 NeuronCores.

Strategy (per core, data-parallel over batch, 8 batches/core):
  1. The synaptic-current recurrence is LINEAR, so it folds into TensorE as
     a [T,T] lower-triangular Toeplitz filter applied to the INPUTS:
       xf = L1s-filter(x);  S[t,k] = xf^T @ (w_eff*j_eff) + L2s @ teacher
     All matmuls run in plain fp32 (4-pass): slower per-op than f32r but
     removes the hi/lo split's Vector/Scalar elementwise cost entirely, and
     the kernel stays DMA-bound.  The weight product w_eff*j_eff runs on
     VectorE over streamed per-batch chunks.
  2. k-mapping k = kc*128 + p: the drive transpose slices hsb contiguously
     ([t, 128k] -> [k, t] per kc chunk), and the spike transpose maps back
     to 512B-contiguous DRAM runs for the output DMA.
  3. Drive is stored t-major [128, T, 64slots] so the scan's per-step ADD
     reads a contiguous [128, 32] slice.
  4. The nonlinear threshold scan runs in the u_t = ab^-t rescaled domain
     (adaptation becomes a pure accumulator Q; membrane carry scaled by
     kappa).  Three VectorE ops per step per batch-group (2 custom DVE +
     1 add).  Q history lands in a linear buffer qh[128, T+1, 32] so spike
     extraction batches 4 steps per op: dq4 = qh[t+1:t+5]-qh[t:t+4] on
     GpSimd, PE-transpose [128,128] -> PSUM, ScalarE Sign(PSUM)->stage.
  5. Batches split in 2 groups of 4: G0's scan is emission-interleaved with
     G1's weight streaming so only G1's scan is an exposed tail.  Spikes
     DMA out from stage in 2 chunks per group.
"""
import numpy as np

import concourse.bass as bass
import concourse.tile as tile
from concourse.bass import _add_dep_helper
from concourse import bacc, mybir
from concourse.bass_utils import run_bass_kernel_spmd

T, B, J, K = 100, 64, 512, 1024
DT = 1.0
NCORES = 8
BLOC = B // NCORES           # 8 batches per core
NKC = 8                      # k chunk count: k = kc*128 + p
NSLOT = BLOC * NKC           # 64 scan slots (b*8 + kc)
NG = 2                       # batch groups
GB = BLOC // NG              # 4 batches per group
GSLOT = NSLOT // NG          # 32 slots per group
F32 = mybir.dt.float32


# ---------------------------------------------------------------------------
# Custom DVE ops (registered into concourse.dve_ops at import time).
def _dve_relu_np(x):
    return np.maximum(np.nan_to_num(x, nan=0.0, posinf=np.inf,
                                    neginf=-np.inf), 0)


def _register_dve(name, spec):
    import concourse.dve_ops as dops
    from concourse.dve_spec import lower, _has_src1
    from concourse.dve_uop import DveOpSpec
    if name in dops._SUB_OPCODE_FOR_NAME:
        return next(op for op in dops.OPS if op.name == name)
    row = dops._CUSTOM_DVE_ROW_BASE + len(dops.OPS)
    assert row < 0x20, "custom DVE row budget exhausted"
    shas = {}
    for ver in ("v3", "v4"):
        s = DveOpSpec(name=name, opcode=row, uops=lower(spec, ver=ver),
                      rd1_en=_has_src1(spec))
        shas[ver] = s.sha(ver)
    op = dops.DveOp(name, spec, subdim=False, uops_sha=shas)
    dops.OPS.append(op)
    dops.CUSTOM_DVE_SPECS[name] = spec
    dops._SUB_OPCODE_FOR_NAME[name] = row
    return op


def _alif_ops():
    from concourse.dve_spec import Spec, Src0, Src1, C0, C1, Zero, relu, select
    carry = _register_dve(
        "ALIF_CARRY",
        Spec(
            body=select(Src0 > Src1, Zero, relu(Src0 + C1) * C0),
            reference=lambda in0, in1, s0, s1, imm2: np.where(
                in0 > in1, np.float32(0.0),
                _dve_relu_np(in0 + s1) * s0).astype(np.float32),
        ),
    )
    qacc = _register_dve(
        "ALIF_QACC",
        Spec(
            body=select(Src0 > Src1, C0, Zero) + Src1,
            reference=lambda in0, in1, s0, s1, imm2: (np.where(
                in0 > in1, np.float32(s0), np.float32(0.0)) + in1
            ).astype(np.float32),
        ),
    )
    return carry, qacc


def _scalar(v, name):
    v = np.asarray(v, np.float64)
    if v.ndim == 0:
        return float(v)
    if np.ptp(v) != 0.0:
        raise NotImplementedError(f"{name} must be uniform for this kernel")
    return float(v.reshape(-1)[0])


def _host_constants(w_teach, tau_mem, tau_adapt, tau_epsp, thr_0, beta_adapt):
    dm = DT / _scalar(tau_mem, "tau_mem")
    dmc = 1.0 - dm
    da = DT / _scalar(tau_adapt, "tau_adapt")
    ab = 1.0 - da
    thr0 = _scalar(thr_0, "thr_0")
    assert thr0 > 0.0, "kernel assumes thr_0 > 0"
    beta = _scalar(beta_adapt, "beta_adapt")
    assert beta > 0.0, "spike extraction via Sign(dq) needs beta_adapt > 0"
    epsp = 1.0 - DT / _scalar(tau_epsp, "tau_epsp")
    wt = _scalar(w_teach, "w_teach")

    u = ab ** (-np.arange(T + 1, dtype=np.float64))      # u_t = ab^-t
    kappa = dmc / ab
    g_bias = thr0 * u[:T]                                # G_t
    c_acc = beta * u[:T]                                 # c_t

    tt_, tau_ = np.meshgrid(np.arange(T), np.arange(T), indexing="ij")
    base = np.where(tau_ <= tt_ - 1,
                    epsp ** np.maximum(tt_ - 1 - tau_, 0), 0.0)
    l1 = (u[:T, None] * dm * base).astype(np.float32)    # [t, tau]
    l2 = (u[:T, None] * dm * wt * base).astype(np.float32)
    l1t = np.ascontiguousarray(l1.T)                     # [tau, t]
    l2t = np.ascontiguousarray(l2.T)

    gcol = (-g_bias).astype(np.float32).reshape(T, 1)    # drive bias: -G_t
    id100 = np.eye(T, dtype=np.float32)
    id128 = np.eye(128, dtype=np.float32)
    return dict(kappa=kappa, g_bias=g_bias, c_acc=c_acc,
                l1t=l1t, l2t=l2t, id100=id100, id128=id128, gcol=gcol)


def build_program(consts):
    """One SPMD program; all 8 cores run it on their own batch shard."""
    kappa = float(consts["kappa"])
    g_bias = consts["g_bias"]
    c_acc = consts["c_acc"]
    CARRY, QACC = _alif_ops()
    nc = bacc.Bacc("TRN2", target_bir_lowering=False, debug=False,
                   num_devices=NCORES)

    x_h = nc.declare_dram_parameter("x", [T, BLOC, J], F32, isOutput=False)
    te_h = nc.declare_dram_parameter("teacher", [T, BLOC, K], F32,
                                     isOutput=False)
    we_h = nc.declare_dram_parameter("w_eff", [BLOC, J, K], F32,
                                     isOutput=False)
    je_h = nc.declare_dram_parameter("j_eff", [BLOC, J, K], F32,
                                     isOutput=False)
    l1_h = nc.declare_dram_parameter("l1t", [T, T], F32, isOutput=False)
    l2_h = nc.declare_dram_parameter("l2t", [T, T], F32, isOutput=False)
    i1_h = nc.declare_dram_parameter("id100", [T, T], F32, isOutput=False)
    i2_h = nc.declare_dram_parameter("id128", [128, 128], F32,
                                     isOutput=False)
    gc_h = nc.declare_dram_parameter("gcol", [T, 1], F32, isOutput=False)
    out_h = nc.declare_dram_parameter("out", [T, BLOC, K], F32, isOutput=True)

    JT = J // 128            # 4 j-tiles

    from contextlib import ExitStack
    with tile.TileContext(nc) as tc, ExitStack() as ctx:
        cpool = ctx.enter_context(tc.tile_pool(name="consts", bufs=1))
        xpool = ctx.enter_context(tc.tile_pool(name="x", bufs=1))
        xfpool = ctx.enter_context(tc.tile_pool(name="xf", bufs=1))
        wpool = ctx.enter_context(tc.tile_pool(name="w", bufs=3))
        jpool = ctx.enter_context(tc.tile_pool(name="j", bufs=2))
        tpool = ctx.enter_context(tc.tile_pool(name="teach", bufs=2))
        hpool = ctx.enter_context(tc.tile_pool(name="h", bufs=2))
        dpool = ctx.enter_context(tc.tile_pool(name="drive", bufs=1))
        qpool = ctx.enter_context(tc.tile_pool(name="qh", bufs=1))
        scpool = ctx.enter_context(tc.tile_pool(name="scan", bufs=2))
        dqpool = ctx.enter_context(tc.tile_pool(name="dq", bufs=2))
        stpool = ctx.enter_context(tc.tile_pool(name="stage", bufs=1))
        ps_h = ctx.enter_context(tc.tile_pool(name="psH", bufs=2,
                                              space="PSUM"))
        ps_t = ctx.enter_context(tc.tile_pool(name="psT", bufs=2,
                                              space="PSUM"))
        ps_s = ctx.enter_context(tc.tile_pool(name="psS", bufs=2,
                                              space="PSUM"))

        # --- prologue DMAs: chained in need-order.  All DMA lanes drain
        # concurrently, so without explicit deps every queued transfer
        # fair-shares bandwidth and COMPLETES late together; chaining batch
        # groups serializes them so completion order == consumption order.
        we_t = [None] * BLOC
        je_t = [None] * BLOC
        te_t = [None] * BLOC
        dma_chain = []            # sliding-window ordering of stream DMAs
        CHAIN_W = 6               # in-flight transfers (~5-6 MB rolling)

        def chained_dma(out, in_):
            h = nc.sync.dma_start(out, in_)
            if len(dma_chain) >= CHAIN_W:
                _add_dep_helper(h.ins, dma_chain[-CHAIN_W].ins, sync=True,
                                reason="dma stream order")
            dma_chain.append(h)
            return h

        def issue_b_dmas(b):
            we_t[b] = wpool.tile([128, JT, K], F32, tag="weff", name="weff")
            je_t[b] = jpool.tile([128, JT, K], F32, tag="jeff", name="jeff")
            te_t[b] = tpool.tile([T, K], F32, tag="teach", name="teach")
            we_src = we_h.ap()[b].rearrange("(jt p) k -> p jt k", p=128)
            je_src = je_h.ap()[b].rearrange("(jt p) k -> p jt k", p=128)
            chained_dma(we_t[b][:, :2], we_src[:, :2])
            chained_dma(je_t[b][:, :2], je_src[:, :2])
            chained_dma(we_t[b][:, 2:], we_src[:, 2:])
            chained_dma(je_t[b][:, 2:], je_src[:, 2:])
            chained_dma(te_t[b][:], te_h.ap()[:, b, :])

        l1t_sb = cpool.tile([T, T], F32, tag="l1")
        l2t_sb = cpool.tile([T, T], F32, tag="l2")
        i1_sb = cpool.tile([T, T], F32, tag="id100")
        i2_sb = cpool.tile([128, 128], F32, tag="id128")
        gc_sb = cpool.tile([T, 1], F32, tag="gc")
        x_sb = xpool.tile([T, BLOC, J], F32, tag="x")
        chained_dma(x_sb[:], x_h.ap()[:])
        chained_dma(l1t_sb[:], l1_h.ap()[:])

        issue_b_dmas(0)

        chained_dma(l2t_sb[:], l2_h.ap()[:])
        chained_dma(i1_sb[:], i1_h.ap()[:])
        chained_dma(i2_sb[:], i2_h.ap()[:])
        chained_dma(gc_sb[:], gc_h.ap()[:])

        issue_b_dmas(1)

        # --- x-filter fold: xf[j, t] = sum_tau x[tau, j] * L1s[t, tau]
        xf_sb = xfpool.tile([128, BLOC * JT, T], F32, tag="xf")
        for b in range(BLOC):
            for jt in range(JT):
                xp = ps_t.tile([128, T], F32, tag="pst")
                nc.tensor.matmul(xp[:],
                                 lhsT=x_sb[:, b, jt * 128:(jt + 1) * 128],
                                 rhs=l1t_sb[:], start=True, stop=True)
                nc.scalar.copy(xf_sb[:, b * JT + jt, :], xp[:])

        # --- drive tiles, t-major, one per scan group so group-1 writes
        # share no tile with group-0 scan reads (no false WAR/RAW deps)
        drive_g = [dpool.tile([128, T, GSLOT], F32, tag=f"drive{g}",
                              name=f"drive{g}") for g in range(NG)]

        # --- scan state (shared across both groups sequentially)
        qh = qpool.tile([128, T + 1, GSLOT], F32, tag="qh")
        stage = [None, None]
        p_prev = [None]

        def emit_spikes(g, t0, nsteps):
            """Extract spikes for steps [t0, t0+nsteps) from the Q history.
            The transpose/Sign run at logically-last priority: the scheduler
            must never queue stream matmuls behind a spike transpose whose
            dq input depends on a late scan step."""
            dq = dqpool.tile([128, 8, GSLOT], F32, tag="dq8", name="dq8")
            nc.vector.tensor_tensor(
                dq[:, :nsteps], qh[:, t0 + 1:t0 + 1 + nsteps, :],
                qh[:, t0:t0 + nsteps, :], mybir.AluOpType.subtract)
            prio = tc.cur_priority
            tc.cur_priority = 1_000_000 + prio
            for h in range(nsteps // 4):
                m = t0 // 4 + h
                sps = ps_s.tile([128, 128], F32, tag="spk")
                nc.tensor.transpose(
                    sps[:], dq[:, h * 4:(h + 1) * 4].rearrange(
                        "p t4 s -> p (t4 s)"), i2_sb[:])
                nc.scalar.activation(stage[g][:, m, :], sps[:],
                                     mybir.ActivationFunctionType.Sign)
            tc.cur_priority = prio

        def emit_scan_step(g, t):
            """One u-domain threshold-scan step for group g."""
            p_in = drive_g[g][:, 0, :] if t == 0 else p_prev[0][:]
            if t < T - 1:
                c_t = scpool.tile([128, GSLOT], F32, tag="C")
                nc.vector._custom_dve(
                    CARRY, out=c_t[:], in0=p_in, in1=qh[:, t, :],
                    s0=kappa, s1=float(g_bias[t]))
            nc.vector._custom_dve(
                QACC, out=qh[:, t + 1, :], in0=p_in, in1=qh[:, t, :],
                s0=float(c_acc[t]))
            if t < T - 1:
                p_new = scpool.tile([128, GSLOT], F32, tag="P")
                nc.vector.tensor_tensor(p_new[:], c_t[:],
                                        drive_g[g][:, t + 1, :],
                                        mybir.AluOpType.add)
                p_prev[0] = p_new
            if t % 8 == 7:
                emit_spikes(g, t - 7, 8)
            elif t == T - 1:
                emit_spikes(g, t - 3, 4)

        def emit_scan_start(g):
            nc.vector.memset(qh[:, 0, :], 0.0)
            stage[g] = stpool.tile([128, T // 4, 128], F32, tag="stage", name="stage")
            p_prev[0] = None

        # out DMA view: t = tq*4 + t4, b = g*4 + bl, k = kc*128 + p
        out_r = out_h.ap().rearrange(
            "(tq t4) (g bl) (kc p) -> t4 g (bl kc) tq p",
            t4=4, bl=GB, p=128)

        def emit_out_dma(g, m0, m1):
            for t4 in range(4):
                chained_dma(out_r[t4, g, :, m0:m1, :],
                            stage[g][t4 * 32:(t4 + 1) * 32, m0:m1, :])

        def emit_b(b, scan_g=None, scan_iter=None):
            """Stream one batch: weight product + matmuls + drive transpose.
            Optionally interleave scan-step emission for group scan_g."""
            if b + 2 < BLOC:
                issue_b_dmas(b + 2)

            def steps(n):
                if scan_iter is None:
                    return
                for _ in range(n):
                    t = next(scan_iter, None)
                    if t is None:
                        return
                    emit_scan_step(scan_g, t)

            # w = w_eff * j_eff in place, 4 chunks of [128, 1024].
            # GpSimd for all but the last batch keeps VectorE free for the
            # scan; the last batch uses VectorE so its drive (which gates the
            # exposed final scan) is not stuck behind GpSimd's queue.
            eng = nc.vector if b < GB else nc.gpsimd
            for jt in range(JT):
                eng.tensor_tensor(
                    we_t[b][:, jt], we_t[b][:, jt], je_t[b][:, jt],
                    mybir.AluOpType.mult)
                steps(7)

            hps = ps_h.tile([T, K], F32, tag="hps")
            hsb = hpool.tile([T, K], F32, tag="hsb")
            for half in range(2):
                ksl = slice(half * 512, (half + 1) * 512)
                for jt in range(JT):
                    nc.tensor.matmul(
                        hps[:, ksl],
                        lhsT=xf_sb[:, b * JT + jt, :],
                        rhs=we_t[b][:, jt, ksl],
                        start=(jt == 0), stop=False)
                nc.tensor.matmul(
                    hps[:, ksl], lhsT=l2t_sb[:], rhs=te_t[b][:, ksl],
                    start=False, stop=True)
                steps(1)
                # d_hat[t] = d[t] - G_t, PSUM -> SBUF with bias
                nc.scalar.activation(hsb[:, ksl], hps[:, ksl],
                                     mybir.ActivationFunctionType.Identity,
                                     bias=gc_sb[:, 0:1], scale=1.0)
                # transpose drive [t, 128k] -> [k, t] per kc chunk
                for kc in range(half * 4, half * 4 + 4):
                    dps = ps_t.tile([128, T], F32, tag="pst")
                    nc.tensor.transpose(
                        dps[:], hsb[:, kc * 128:(kc + 1) * 128], i1_sb[:])
                    nc.scalar.copy(
                        drive_g[b // GB][:, :, (b % GB) * NKC + kc], dps[:])
                    steps(1)
            we_t[b] = None
            je_t[b] = None
            te_t[b] = None

        # --- phase A: group 0 batches, no scan yet
        for b in range(GB):
            emit_b(b)

        # --- phase B: group 1 batches with group-0 scan interleaved
        emit_scan_start(0)
        it0 = iter(range(T))
        for b in range(GB, BLOC):
            emit_b(b, scan_g=0, scan_iter=it0)
        for t in it0:
            emit_scan_step(0, t)
        emit_out_dma(0, 0, 13)
        emit_out_dma(0, 13, T // 4)

        # --- phase C: group-1 scan + outputs
        emit_scan_start(1)
        for t in range(T):
            emit_scan_step(1, t)
            if t == 60:
                emit_out_dma(1, 0, 13)
        emit_out_dma(1, 13, T // 4)

    nc.compile()
    return nc


def _prepare(inputs):
    x = np.ascontiguousarray(np.asarray(inputs["x"], np.float32))
    teacher = np.ascontiguousarray(np.asarray(inputs["teacher"], np.float32))
    w_eff = np.ascontiguousarray(np.asarray(inputs["w_eff"], np.float32))
    j_eff = np.ascontiguousarray(np.asarray(inputs["j_eff"], np.float32))
    consts = _host_constants(
        inputs["w_teach"], inputs["tau_mem"], inputs["tau_adapt"],
        inputs["tau_epsp"], inputs["thr_0"], inputs["beta_adapt"])
    in_maps = []
    for i in range(NCORES):
        sl = slice(i * BLOC, (i + 1) * BLOC)
        in_maps.append({
            "x": np.ascontiguousarray(x[:, sl]),
            "teacher": np.ascontiguousarray(teacher[:, sl]),
            "w_eff": np.ascontiguousarray(w_eff[sl]),
            "j_eff": np.ascontiguousarray(j_eff[sl]),
            "l1t": consts["l1t"], "l2t": consts["l2t"],
            "id100": consts["id100"], "id128": consts["id128"],
            "gcol": consts["gcol"],
        })
    return consts, in_maps


def run(inputs, trace=False, **kw):
    consts, in_maps = _prepare(inputs)
    nc = build_program(consts)
    res = run_bass_kernel_spmd(nc, in_maps, core_ids=list(range(NCORES)),
                               trace=trace, **kw)
    out = np.concatenate([res.results[i]["out"] for i in range(NCORES)],
                         axis=1)
    return out.astype(np.float32), res


def kernel(**inputs) -> np.ndarray:
    out, _ = run(inputs)
    return out
